# revision 4
# baseline (speedup 1.0000x reference)
"""BiLevelRoutingAttention Trainium2 kernel (v3).

The Tensor-queue is the bottleneck: span ~= sum of LDWEIGHTS+issue per
matmul (~98ns each in v2, 3123 matmuls -> 305us).  v3 cuts the matmul
count per (b,t) tile from 82 to 58 and the routing preamble from ~264
to ~40 matmuls per batch:

  - qk projection pair-batched over 2 tiles (N=512): 8 -> 4 MM/tile.
  - out projection feature-major + pair-batched:      4 -> 2 MM/tile.
  - V bias folded into the proj bias on host (out = (PV + bv*Z)/Z @ Wp
    + bp == atn @ Wp + (bp + bv@Wp)): bias matmuls gone.
  - mask add merged over both key halves (K=16 stationary, N=512
    two-block e8r2 constant): 16 -> 8 MM/tile.
  - Z via col-tiled ones (M=32) with N=512 (both key halves in free),
    halves summed on DVE: 16 -> 8 MM/tile.
  - routing sim as block-diagonal fp32 matmuls (M=128 covering all 4
    heads x (2 dup x 8 qwin)): 256 -> 32 MM/batch, and the duplicated
    rows directly provide the K=16 merged-mask stationary layout.
  - mask window-expansion done once per batch by DMA (SBUF->SBUF
    broadcast reads), freeing gpsimd and the per-tile critical path.
  - bf16 output, halves the store DMA.
"""

import sys

sys.path.insert(0, "/opt/trn_rl_repo")

import numpy as np
import ml_dtypes

import concourse.bass as bass
import concourse.bacc as bacc
import concourse.mybir as mybir
import concourse.tile as tile
from concourse.bass_utils import run_bass_kernel_spmd

BF16 = mybir.dt.bfloat16
F32 = mybir.dt.float32

NCORES = 8
B, T, S, C = 16, 16, 256, 256
NW, WIN, NH, D, TK = 8, 32, 8, 32, 4
BPC = B // NCORES  # batches per core
NP = T // 2        # tile pairs per batch
SCALE = float(D) ** -0.5
MASKVAL = -1e9

_CACHE = {}


def _build_nc(nt=T):
    nc = bacc.Bacc("TRN2", target_bir_lowering=False, debug=False)
    AL = mybir.AluOpType
    ACTF = mybir.ActivationFunctionType

    xt_d = nc.dram_tensor("xt", [BPC, nt, C, S], BF16, kind="ExternalInput")
    xs_d = nc.dram_tensor("xsumt", [BPC, C, nt, NW], F32, kind="ExternalInput")
    wqk_d = nc.dram_tensor("wqk_bf", [C, 2 * C], BF16, kind="ExternalInput")
    wqkf_d = nc.dram_tensor("wqk_f32", [C, 2 * C], F32, kind="ExternalInput")
    wv_d = nc.dram_tensor("wv_bf", [C, C], BF16, kind="ExternalInput")
    wp_d = nc.dram_tensor("wproj_bf", [C, C], BF16, kind="ExternalInput")
    bqk_d = nc.dram_tensor("bqk_cols", [128, 4], F32, kind="ExternalInput")
    bqkr_d = nc.dram_tensor("bqk_reg", [128, 4], F32, kind="ExternalInput")
    bp_d = nc.dram_tensor("bp_col", [128, 2], F32, kind="ExternalInput")
    e8r2_d = nc.dram_tensor("e8r2", [128, 2 * S], BF16, kind="ExternalInput")
    # out: [b, pair, feat_part, jb, (ti,s)] bf16 (feature-major)
    out_d = nc.dram_tensor("out", [BPC, NP, 128, 2, 2 * S], BF16,
                           kind="ExternalOutput")

    with tile.TileContext(nc) as tc:
        with (
            tc.tile_pool(name="wpool", bufs=1) as wp,
            tc.tile_pool(name="route", bufs=1) as rp,
            tc.tile_pool(name="xpool", bufs=3) as xp,
            tc.tile_pool(name="qkpool", bufs=2) as qp,
            tc.tile_pool(name="vpool", bufs=3) as vp,
            tc.tile_pool(name="exps", bufs=2) as ep,
            tc.tile_pool(name="zpool", bufs=2) as zp,
            tc.tile_pool(name="apool", bufs=2) as ap_,
            tc.tile_pool(name="opool", bufs=2) as op_,
            tc.tile_pool(name="sc", bufs=2, space="PSUM") as psc,
            tc.tile_pool(name="p1", bufs=4, space="PSUM") as pp1,
        ):
            # ---- weights / constants (loaded once) ----
            wqk_sb = wp.tile([128, 2, 2 * C], BF16)
            nc.sync.dma_start(out=wqk_sb, in_=wqk_d.ap().rearrange("(cc p) j -> p cc j", p=128))
            wqkf_sb = wp.tile([128, 2, 2 * C], F32)
            nc.sync.dma_start(out=wqkf_sb, in_=wqkf_d.ap().rearrange("(cc p) j -> p cc j", p=128))
            wv_sb = wp.tile([128, 2, C], BF16)
            nc.sync.dma_start(out=wv_sb, in_=wv_d.ap().rearrange("(cc p) j -> p cc j", p=128))
            wp_sb = wp.tile([128, 2, C], BF16)
            nc.sync.dma_start(out=wp_sb, in_=wp_d.ap().rearrange("(cc p) j -> p cc j", p=128))
            bqk_sb = wp.tile([128, 4], F32)
            nc.sync.dma_start(out=bqk_sb, in_=bqk_d.ap())
            bqkr_sb = wp.tile([128, 4], F32)
            nc.sync.dma_start(out=bqkr_sb, in_=bqkr_d.ap())
            bp_sb = wp.tile([128, 2], F32)
            nc.sync.dma_start(out=bp_sb, in_=bp_d.ap())
            e8r2_sb = wp.tile([128, 2 * S], BF16)
            nc.sync.dma_start(out=e8r2_sb, in_=e8r2_d.ap())
            ones32_sb = wp.tile([128, 32], BF16)
            nc.vector.memset(ones32_sb, 1.0)

            # PE warm-up: back-to-back junk matmuls un-throttle the HAM
            # clock gate before the (serial) routing preamble
            warm_ps = psc.tile([128, 2, 2 * S], F32, tag="sc")
            for w in range(20):
                nc.tensor.matmul(warm_ps[:, 0, :],
                                 lhsT=wqk_sb[:, 0, 0:128],
                                 rhs=wqk_sb[:, 1, :],
                                 start=(w == 0), stop=(w == 19))

            # ================= routing preamble (per batch) =================
            def routing(b):
                xs_sb = rp.tile([128, 2, nt * NW], F32, tag=f"xsb{b}")
                nc.sync.dma_start(
                    out=xs_sb,
                    in_=xs_d[b].rearrange("(cc p) t n -> p cc (t n)", p=128))

                # region features: [q;k]^T = Wqk^T @ xsum  (exact fp32)
                rs_ps = pp1.tile([128, 4, nt * NW], F32, tag="p1")
                for jb in range(4):
                    for cc in range(2):
                        nc.tensor.matmul(rs_ps[:, jb, :],
                                         lhsT=wqkf_sb[:, cc, jb * 128:(jb + 1) * 128],
                                         rhs=xs_sb[:, cc, :],
                                         start=(jb == 0 and cc == 0),
                                         stop=(jb == 3 and cc == 1))
                rgs_sb = rp.tile([128, 4, nt * NW], F32, tag=f"rgs{b}")
                nc.vector.tensor_tensor(
                    out=rgs_sb, in0=rs_ps,
                    in1=bqkr_sb[:].unsqueeze(-1).to_broadcast([128, 4, nt * NW]),
                    op=AL.add)

                # block-diagonal q-region stationary: ddiag[32rg+d, jbq, t,
                # 32rg + 8u + qwin] = qreg_rg[d, qwin] (u-duplicated),
                # zeros elsewhere -> one M=128 fp32 matmul per (jbq, t)
                # computes all 4 heads' 8x8 sims with 2x row duplication.
                ddiag = rp.tile([128, 2, nt, 128], F32, tag=f"ddiag{b}")
                nc.vector.memset(ddiag, 0.0)
                for jbq in range(2):
                    for rg in range(4):
                        src = rgs_sb[32 * rg:32 * rg + 32, jbq, :] \
                            .rearrange("p (t n) -> p t n", n=NW) \
                            .unsqueeze(2).to_broadcast([32, nt, 2, NW])
                        nc.vector.tensor_copy(
                            out=ddiag[32 * rg:32 * rg + 32, jbq, :,
                                      32 * rg:32 * rg + 16]
                                .rearrange("p t (u n) -> p t u n", n=NW),
                            in_=src)

                sim_ps = pp1.tile([128, 2, nt * NW], F32, tag="p1")
                for jbq in range(2):
                    for t in range(nt):
                        nc.tensor.matmul(
                            sim_ps[:, jbq, t * NW:(t + 1) * NW],
                            lhsT=ddiag[:, jbq, t, :],
                            rhs=rgs_sb[:, 2 + jbq, t * NW:(t + 1) * NW],
                            start=True, stop=True)
                sim_sb = rp.tile([128, 2, nt * NW], F32, tag=f"sim{b}")
                nc.vector.tensor_copy(out=sim_sb, in_=sim_ps)

                # top-4 threshold per (head, qwin): 4th largest of the 8
                mx_sb = rp.tile([128, 2, nt * NW], F32, tag=f"mx{b}")
                mw_sb = rp.tile([128, 2, nt * NW], BF16, tag=f"mw{b}")
                for t in range(nt):
                    for jbq in range(2):
                        nc.vector.max(
                            out=mx_sb[:, jbq, t * NW:(t + 1) * NW],
                            in_=sim_sb[:, jbq, t * NW:(t + 1) * NW])
                for jbq in range(2):
                    nc.vector.tensor_tensor(
                        out=mw_sb[:, jbq, :]
                            .rearrange("p (t n) -> p t n", n=NW),
                        in0=sim_sb[:, jbq, :]
                            .rearrange("p (t n) -> p t n", n=NW),
                        in1=mx_sb[:, jbq, 3::NW]
                            .unsqueeze(-1).to_broadcast([128, nt, NW]),
                        op=AL.is_ge)
                # mask = (m01 - 1) * 1e9  ->  0 selected / -1e9 dropped
                nc.vector.tensor_scalar(
                    out=mw_sb, in0=mw_sb,
                    scalar1=1.0, scalar2=-MASKVAL,
                    op0=AL.subtract, op1=AL.mult)

                # window-expand the mask for all tiles: first compact the
                # 4 relevant kwins per u-row-group (8 tiny DVE copies),
                # then one DMA broadcast-read expands kwin -> 32 keys:
                # mwx[32rg+8u+qwin, jbq, t, k] =
                #   mw[32rg+8u+qwin, jbq, t*8 + 4u + k//32]
                mw2_sb = rp.tile([128, 2, nt, 4], BF16, tag=f"mw2{b}")
                nc.vector.memset(mw2_sb, 0.0)
                for rg in range(4):
                    for u in range(2):
                        r0 = 32 * rg + 8 * u
                        for jbq in range(2):
                            nc.sync.dma_start(
                                out=mw2_sb[r0:r0 + 8, jbq, :, :],
                                in_=mw_sb[r0:r0 + 8, jbq, :]
                                    .rearrange("p (t n) -> p t n", n=NW)[
                                        :, :, 4 * u:4 * u + 4])
                mwx_sb = rp.tile([128, 2, nt, 128], BF16, tag=f"mwx{b}")
                nc.vector.tensor_copy(
                    out=mwx_sb.rearrange("p j t (n w) -> p (j t n) w", w=WIN),
                    in_=mw2_sb.rearrange("p j t n -> p (j t n)")
                        .unsqueeze(-1).to_broadcast([128, 2 * nt * 4, WIN]))
                return mwx_sb

            # ================= main loop: 1-pair software pipeline ==========
            # producer(p): xt DMA + qk pair + v tiles for pair p
            # consumer(p): scores/exp/Z/PV/proj/out for pair p
            mwx_sbs = [None] * BPC
            qk_sbs = {}
            v_sbs = {}

            def producer(p, b):
                xt_sb = xp.tile([128, 2, 2, S], BF16, tag="xt")
                for ti in range(2):
                    nc.sync.dma_start(
                        out=xt_sb[:, :, ti, :],
                        in_=xt_d[b, 2 * p + ti].rearrange(
                            "(cc q) s -> q cc s", q=128))

                # qk pair (feature-major): [128, 4jb, 512] bf16
                qk_sb = qp.tile([128, 4, 2 * S], BF16, tag="qk")
                for jb in range(4):
                    qps = pp1.tile([128, 2 * S], F32, tag="p1")
                    for cc in range(2):
                        nc.tensor.matmul(
                            qps,
                            lhsT=wqk_sb[:, cc, jb * 128:(jb + 1) * 128],
                            rhs=xt_sb[:, cc, :, :],
                            start=(cc == 0), stop=(cc == 1))
                    nc.vector.tensor_tensor(
                        out=qk_sb[:, jb, :], in0=qps,
                        in1=bqk_sb[:, jb].unsqueeze(-1).to_broadcast([128, 2 * S]),
                        op=AL.add)
                qk_sbs[(p, b)] = qk_sb

                # V (token-major) per tile
                for ti in range(2):
                    v_sb = vp.tile([128, 2, C], BF16, tag="v")
                    vps = pp1.tile([128, 2, C], F32, tag="p1")
                    for sb_ in range(2):
                        for cc in range(2):
                            nc.tensor.matmul(
                                vps[:, sb_, :],
                                lhsT=xt_sb[:, cc, ti,
                                           sb_ * 128:sb_ * 128 + 128],
                                rhs=wv_sb[:, cc, :],
                                start=(sb_ == 0 and cc == 0),
                                stop=(sb_ == 1 and cc == 1))
                    nc.vector.tensor_copy(out=v_sb, in_=vps)
                    v_sbs[(2 * p + ti, b)] = v_sb

            def consumer(p, b):
                mwx_sb = mwx_sbs[b]
                qk_sb = qk_sbs.pop((p, b))
                atn_sb = ap_.tile([128, 2, 2, S], BF16, tag="atn")
                for ti in range(2):
                    t = 2 * p + ti
                    v_sb = v_sbs.pop((t, b))
                    toff = ti * S

                    # ---- scores^T + merged mask, exp ----
                    expT = ep.tile([128, 2, 4, 2 * S], BF16, tag="expT")
                    for jbq in range(2):
                        for rpp in range(2):
                            sc_ps = psc.tile([128, 2, 2 * S], F32, tag="sc")
                            for rr in range(2):
                                rg = 2 * rpp + rr
                                for kb in range(2):
                                    nc.tensor.matmul(
                                        sc_ps[:, rr, kb * S:(kb + 1) * S],
                                        lhsT=qk_sb[32 * rg:32 * rg + 32, 2 + jbq,
                                                   toff + kb * 128:toff + kb * 128 + 128],
                                        rhs=qk_sb[32 * rg:32 * rg + 32, jbq,
                                                  toff:toff + S],
                                        start=(kb == 0), stop=False,
                                        skip_group_check=True,
                                        tile_position=(32 * rg, 0))
                                nc.tensor.matmul(
                                    sc_ps[:, rr, :],
                                    lhsT=mwx_sb[32 * rg:32 * rg + 16, jbq, t, :],
                                    rhs=e8r2_sb[32 * rg:32 * rg + 16, :],
                                    start=False, stop=True,
                                    skip_group_check=True,
                                    tile_position=(32 * rg, 0))
                            nc.scalar.activation(
                                out=expT[:, jbq, 2 * rpp:2 * rpp + 2, :],
                                in_=sc_ps, func=ACTF.Exp, scale=SCALE)

                    # ---- Z: pre-sum exp key-halves (SBUF bf16 TT), then
                    # col-tiled ones matmuls replicate Z onto each head's
                    # 32 partitions; reciprocal reads PSUM directly ----
                    esum_sb = zp.tile([128, 2, 4, S], BF16, tag="esum")
                    nc.vector.tensor_tensor(
                        out=esum_sb, in0=expT[:, :, :, 0:S],
                        in1=expT[:, :, :, S:2 * S], op=AL.add)
                    zrep = pp1.tile([128, 2, S], F32, tag="p1")
                    for jbq in range(2):
                        for rg in range(4):
                            nc.tensor.matmul(
                                zrep[32 * rg:32 * rg + 32, jbq, :],
                                lhsT=ones32_sb,
                                rhs=esum_sb[:, jbq, rg, :],
                                start=True, stop=True,
                                skip_group_check=True,
                                tile_position=(0, 32 * rg))
                    zinv_sb = zp.tile([128, 2, S], F32, tag="zinv")
                    nc.vector.reciprocal_approx_fast(out=zinv_sb, in_=zrep)

                    # ---- PV (col-packed, both quads in one bank) ----
                    at = pp1.tile([128, 2, S], F32, tag="p1")
                    for jbq in range(2):
                        for rg in range(4):
                            hh = 4 * jbq + rg
                            for kb in range(2):
                                nc.tensor.matmul(
                                    at[32 * rg:32 * rg + 32, jbq, :],
                                    lhsT=v_sb[:, kb, 32 * hh:32 * hh + 32],
                                    rhs=expT[:, jbq, rg, kb * S:(kb + 1) * S],
                                    start=(jbq == 0 and kb == 0),
                                    stop=(jbq == 1 and kb == 1),
                                    skip_group_check=True,
                                    tile_position=(0, 32 * rg))
                    nc.vector.tensor_tensor(out=atn_sb[:, :, ti, :], in0=at,
                                            in1=zinv_sb, op=AL.mult)

                # ---- out projection (feature-major, pair-batched) ----
                out_sb = op_.tile([128, 2, 2 * S], BF16, tag="out")
                for jb in range(2):
                    po = pp1.tile([128, 2 * S], F32, tag="p1")
                    for cc in range(2):
                        nc.tensor.matmul(
                            po,
                            lhsT=wp_sb[:, cc, jb * 128:(jb + 1) * 128],
                            rhs=atn_sb[:, cc, :, :],
                            start=(cc == 0), stop=(cc == 1))
                    nc.vector.tensor_tensor(
                        out=out_sb[:, jb, :], in0=po,
                        in1=bp_sb[:, jb].unsqueeze(-1).to_broadcast([128, 2 * S]),
                        op=AL.add)
                nc.sync.dma_start(out=out_d[b, p], in_=out_sb)

            # pipeline: producer one pair ahead of consumer; batch-1
            # routing overlaps batch-0's first tiles
            for b in range(BPC):
                mwx_sbs[b] = routing(b)
                producer(0, b)
            for p in range(NP):
                for b in range(BPC):
                    if p + 1 < NP:
                        producer(p + 1, b)
                    consumer(p, b)

    nc.compile()
    return nc


def _host_prep(x, w_qkv, b_qkv, w_proj, b_proj):
    bf16 = ml_dtypes.bfloat16
    x4 = x.reshape(B, T, S, C)
    xt = np.ascontiguousarray(x4.transpose(0, 1, 3, 2)).astype(bf16)
    xsum = x4.reshape(B, T, NW, WIN, C).sum(3, dtype=np.float64).astype(np.float32)
    xsumt = np.ascontiguousarray(xsum.transpose(0, 3, 1, 2))  # [B, C, T, NW]

    # fold v bias through the projection: out = atn@Wp + (bp + bv@Wp)
    bp_eff = (b_proj + b_qkv[2 * C:] @ w_proj).astype(np.float32)

    shared = {
        "wqk_bf": np.ascontiguousarray(w_qkv[:, :2 * C]).astype(bf16),
        "wqk_f32": np.ascontiguousarray(w_qkv[:, :2 * C]).astype(np.float32),
        "wv_bf": np.ascontiguousarray(w_qkv[:, 2 * C:]).astype(bf16),
        "wproj_bf": w_proj.astype(bf16),
        "bqk_cols": np.ascontiguousarray(
            b_qkv[:2 * C].reshape(4, 128).T).astype(np.float32),
        "bqk_reg": np.ascontiguousarray(
            (WIN * b_qkv[:2 * C]).reshape(4, 128).T).astype(np.float32),
        "bp_col": np.ascontiguousarray(bp_eff.reshape(2, 128).T),
        "e8r2": _make_e8r2(),
    }
    in_maps = []
    for core in range(NCORES):
        b0 = core * BPC
        m = dict(shared)
        m["xt"] = np.ascontiguousarray(xt[b0:b0 + BPC])
        m["xsumt"] = np.ascontiguousarray(xsumt[b0:b0 + BPC])
        in_maps.append(m)
    return in_maps


def _make_e8r2():
    e = np.zeros((128, 2 * S), ml_dtypes.bfloat16)
    q = np.arange(S) // WIN  # query window of column q
    for rg in range(4):
        for u in range(2):
            for w in range(NW):
                e[32 * rg + 8 * u + w, u * S:(u + 1) * S][q == w] = 1.0
    return e


def kernel(x, w_qkv, b_qkv, w_proj, b_proj, **_unused_scalars):
    x = np.asarray(x, dtype=np.float32)
    w_qkv = np.asarray(w_qkv, dtype=np.float32)
    b_qkv = np.asarray(b_qkv, dtype=np.float32)
    w_proj = np.asarray(w_proj, dtype=np.float32)
    b_proj = np.asarray(b_proj, dtype=np.float32)

    if "nc" not in _CACHE:
        _CACHE["nc"] = _build_nc()
    nc = _CACHE["nc"]

    in_maps = _host_prep(x, w_qkv, b_qkv, w_proj, b_proj)
    res = run_bass_kernel_spmd(nc, in_maps, core_ids=list(range(NCORES)))

    out = np.empty((B, NP, 128, 2, 2 * S), np.float32)
    for core in range(NCORES):
        out[core * BPC:(core + 1) * BPC] = res.results[core]["out"]
    # [B, pair, p128, jb, (ti s)] -> [B, N, C]: token = pair*512 + tis,
    # feature c = jb*128 + p128
    out = out.transpose(0, 1, 4, 3, 2).reshape(B, T * S, C)
    return np.ascontiguousarray(out)


# revision 5
# speedup vs baseline: 1.2102x; 1.2102x over previous
"""BiLevelRoutingAttention Trainium2 kernel (v3).

The Tensor-queue is the bottleneck: span ~= sum of LDWEIGHTS+issue per
matmul (~98ns each in v2, 3123 matmuls -> 305us).  v3 cuts the matmul
count per (b,t) tile from 82 to 58 and the routing preamble from ~264
to ~40 matmuls per batch:

  - qk projection pair-batched over 2 tiles (N=512): 8 -> 4 MM/tile.
  - out projection feature-major + pair-batched:      4 -> 2 MM/tile.
  - V bias folded into the proj bias on host (out = (PV + bv*Z)/Z @ Wp
    + bp == atn @ Wp + (bp + bv@Wp)): bias matmuls gone.
  - mask add merged over both key halves (K=16 stationary, N=512
    two-block e8r2 constant): 16 -> 8 MM/tile.
  - Z via col-tiled ones (M=32) with N=512 (both key halves in free),
    halves summed on DVE: 16 -> 8 MM/tile.
  - routing sim as block-diagonal fp32 matmuls (M=128 covering all 4
    heads x (2 dup x 8 qwin)): 256 -> 32 MM/batch, and the duplicated
    rows directly provide the K=16 merged-mask stationary layout.
  - mask window-expansion done once per batch by DMA (SBUF->SBUF
    broadcast reads), freeing gpsimd and the per-tile critical path.
  - bf16 output, halves the store DMA.
"""

import sys

sys.path.insert(0, "/opt/trn_rl_repo")

import numpy as np
import ml_dtypes

import concourse.bass as bass
import concourse.bacc as bacc
import concourse.mybir as mybir
import concourse.tile as tile
from concourse.bass_utils import run_bass_kernel_spmd

BF16 = mybir.dt.bfloat16
F32 = mybir.dt.float32

NCORES = 8
B, T, S, C = 16, 16, 256, 256
NW, WIN, NH, D, TK = 8, 32, 8, 32, 4
BPC = B // NCORES  # batches per core
NP = T // 2        # tile pairs per batch
SCALE = float(D) ** -0.5
MASKVAL = -1e9

_CACHE = {}


def _build_nc(nt=T):
    nc = bacc.Bacc("TRN2", target_bir_lowering=False, debug=False)
    AL = mybir.AluOpType
    ACTF = mybir.ActivationFunctionType

    xt_d = nc.dram_tensor("xt", [BPC, nt, C, S], BF16, kind="ExternalInput")
    xs_d = nc.dram_tensor("xsumt", [BPC, C, nt, NW], F32, kind="ExternalInput")
    wqk_d = nc.dram_tensor("wqk_bf", [C, 2 * C], BF16, kind="ExternalInput")
    wqkf_d = nc.dram_tensor("wqk_f32", [C, 2 * C], F32, kind="ExternalInput")
    wv_d = nc.dram_tensor("wv_bf", [C, C], BF16, kind="ExternalInput")
    wp_d = nc.dram_tensor("wproj_bf", [C, C], BF16, kind="ExternalInput")
    bqk_d = nc.dram_tensor("bqk_cols", [128, 4], F32, kind="ExternalInput")
    bqkr_d = nc.dram_tensor("bqk_reg", [128, 4], F32, kind="ExternalInput")
    bp_d = nc.dram_tensor("bp_col", [128, 2], F32, kind="ExternalInput")
    e8r2_d = nc.dram_tensor("e8r2", [128, 2 * S], BF16, kind="ExternalInput")
    # out: [b, pair, feat_part, jb, (ti,s)] bf16 (feature-major)
    out_d = nc.dram_tensor("out", [BPC, NP, 128, 2, 2 * S], BF16,
                           kind="ExternalOutput")

    with tile.TileContext(nc) as tc:
        with (
            tc.tile_pool(name="wpool", bufs=1) as wp,
            tc.tile_pool(name="route", bufs=1) as rp,
            tc.tile_pool(name="xpool", bufs=6) as xp,
            tc.tile_pool(name="qkpool", bufs=4) as qp,
            tc.tile_pool(name="vpool", bufs=8) as vp,
            tc.tile_pool(name="exps", bufs=4) as ep,
            tc.tile_pool(name="zpool", bufs=2) as zp,
            tc.tile_pool(name="apool", bufs=2) as ap_,
            tc.tile_pool(name="opool", bufs=2) as op_,
            tc.tile_pool(name="sc", bufs=2, space="PSUM") as psc,
            tc.tile_pool(name="p1", bufs=4, space="PSUM") as pp1,
        ):
            # PE warm-up first, against junk (no DMA dependency): ramps the
            # HAM clock gate while the weight DMAs are still in flight
            junk_sb = wp.tile([128, 2 * S], BF16)
            nc.vector.memset(junk_sb, 0.5)
            warm_ps = psc.tile([128, 2, 2 * S], F32, tag="sc")
            for w in range(24):
                nc.tensor.matmul(warm_ps[:, 0, :],
                                 lhsT=junk_sb[:, 0:128],
                                 rhs=junk_sb,
                                 start=(w == 0), stop=(w == 23))

            # ---- weights / constants (loaded once) ----
            wqk_sb = wp.tile([128, 2, 2 * C], BF16)
            nc.sync.dma_start(out=wqk_sb, in_=wqk_d.ap().rearrange("(cc p) j -> p cc j", p=128))
            wqkf_sb = wp.tile([128, 2, 2 * C], F32)
            nc.sync.dma_start(out=wqkf_sb, in_=wqkf_d.ap().rearrange("(cc p) j -> p cc j", p=128))
            wv_sb = wp.tile([128, 2, C], BF16)
            nc.sync.dma_start(out=wv_sb, in_=wv_d.ap().rearrange("(cc p) j -> p cc j", p=128))
            wp_sb = wp.tile([128, 2, C], BF16)
            nc.sync.dma_start(out=wp_sb, in_=wp_d.ap().rearrange("(cc p) j -> p cc j", p=128))
            bqk_sb = wp.tile([128, 4], F32)
            nc.sync.dma_start(out=bqk_sb, in_=bqk_d.ap())
            bqkr_sb = wp.tile([128, 4], F32)
            nc.sync.dma_start(out=bqkr_sb, in_=bqkr_d.ap())
            bp_sb = wp.tile([128, 2], F32)
            nc.sync.dma_start(out=bp_sb, in_=bp_d.ap())
            e8r2_sb = wp.tile([128, 2 * S], BF16)
            nc.sync.dma_start(out=e8r2_sb, in_=e8r2_d.ap())
            ones32_sb = wp.tile([128, 32], BF16)
            nc.vector.memset(ones32_sb, 1.0)

            # ================= routing preamble (per batch) =================
            def routing(b):
                xs_sb = rp.tile([128, 2, nt * NW], F32, tag=f"xsb{b}")
                nc.sync.dma_start(
                    out=xs_sb,
                    in_=xs_d[b].rearrange("(cc p) t n -> p cc (t n)", p=128))

                # region features: [q;k]^T = Wqk^T @ xsum  (exact fp32)
                rs_ps = pp1.tile([128, 4, nt * NW], F32, tag="p1")
                for jb in range(4):
                    for cc in range(2):
                        nc.tensor.matmul(rs_ps[:, jb, :],
                                         lhsT=wqkf_sb[:, cc, jb * 128:(jb + 1) * 128],
                                         rhs=xs_sb[:, cc, :],
                                         start=(jb == 0 and cc == 0),
                                         stop=(jb == 3 and cc == 1))
                rgs_sb = rp.tile([128, 4, nt * NW], F32, tag=f"rgs{b}")
                nc.vector.tensor_tensor(
                    out=rgs_sb, in0=rs_ps,
                    in1=bqkr_sb[:].unsqueeze(-1).to_broadcast([128, 4, nt * NW]),
                    op=AL.add)

                # block-diagonal q-region stationary: ddiag[32rg+d, jbq, t,
                # 32rg + 8u + qwin] = qreg_rg[d, qwin] (u-duplicated),
                # zeros elsewhere -> one M=128 fp32 matmul per (jbq, t)
                # computes all 4 heads' 8x8 sims with 2x row duplication.
                ddiag = rp.tile([128, 2, nt, 128], F32, tag=f"ddiag{b}")
                nc.vector.memset(ddiag, 0.0)
                for jbq in range(2):
                    for rg in range(4):
                        src = rgs_sb[32 * rg:32 * rg + 32, jbq, :] \
                            .rearrange("p (t n) -> p t n", n=NW) \
                            .unsqueeze(2).to_broadcast([32, nt, 2, NW])
                        nc.vector.tensor_copy(
                            out=ddiag[32 * rg:32 * rg + 32, jbq, :,
                                      32 * rg:32 * rg + 16]
                                .rearrange("p t (u n) -> p t u n", n=NW),
                            in_=src)

                sim_ps = pp1.tile([128, 2, nt * NW], F32, tag="p1")
                for jbq in range(2):
                    for t in range(nt):
                        nc.tensor.matmul(
                            sim_ps[:, jbq, t * NW:(t + 1) * NW],
                            lhsT=ddiag[:, jbq, t, :],
                            rhs=rgs_sb[:, 2 + jbq, t * NW:(t + 1) * NW],
                            start=True, stop=True)
                sim_sb = rp.tile([128, 2, nt * NW], F32, tag=f"sim{b}")
                nc.vector.tensor_copy(out=sim_sb, in_=sim_ps)

                # top-4 threshold per (head, qwin): 4th largest of the 8
                mx_sb = rp.tile([128, 2, nt * NW], F32, tag=f"mx{b}")
                mw_sb = rp.tile([128, 2, nt * NW], BF16, tag=f"mw{b}")
                for t in range(nt):
                    for jbq in range(2):
                        nc.vector.max(
                            out=mx_sb[:, jbq, t * NW:(t + 1) * NW],
                            in_=sim_sb[:, jbq, t * NW:(t + 1) * NW])
                for jbq in range(2):
                    nc.vector.tensor_tensor(
                        out=mw_sb[:, jbq, :]
                            .rearrange("p (t n) -> p t n", n=NW),
                        in0=sim_sb[:, jbq, :]
                            .rearrange("p (t n) -> p t n", n=NW),
                        in1=mx_sb[:, jbq, 3::NW]
                            .unsqueeze(-1).to_broadcast([128, nt, NW]),
                        op=AL.is_ge)
                # mask = (m01 - 1) * 1e9  ->  0 selected / -1e9 dropped
                nc.vector.tensor_scalar(
                    out=mw_sb, in0=mw_sb,
                    scalar1=1.0, scalar2=-MASKVAL,
                    op0=AL.subtract, op1=AL.mult)

                # window-expand the mask for all tiles: first compact the
                # 4 relevant kwins per u-row-group (8 tiny DVE copies),
                # then one DMA broadcast-read expands kwin -> 32 keys:
                # mwx[32rg+8u+qwin, jbq, t, k] =
                #   mw[32rg+8u+qwin, jbq, t*8 + 4u + k//32]
                mw2_sb = rp.tile([128, 2, nt, 4], BF16, tag=f"mw2{b}")
                nc.vector.memset(mw2_sb, 0.0)
                # 32 tiny SBUF->SBUF DMAs, spread over two otherwise-idle
                # engine queues so they drain in parallel
                for rg in range(4):
                    for u in range(2):
                        r0 = 32 * rg + 8 * u
                        for jbq in range(2):
                            eng = nc.scalar if (rg + u + jbq) % 2 else nc.gpsimd
                            eng.dma_start(
                                out=mw2_sb[r0:r0 + 8, jbq, :, :],
                                in_=mw_sb[r0:r0 + 8, jbq, :]
                                    .rearrange("p (t n) -> p t n", n=NW)[
                                        :, :, 4 * u:4 * u + 4])
                mwx_sb = rp.tile([128, 2, nt, 128], BF16, tag=f"mwx{b}")
                nc.vector.tensor_copy(
                    out=mwx_sb.rearrange("p j t (n w) -> p (j t n) w", w=WIN),
                    in_=mw2_sb.rearrange("p j t n -> p (j t n)")
                        .unsqueeze(-1).to_broadcast([128, 2 * nt * 4, WIN]))
                return mwx_sb

            # ================= main loop: 1-pair software pipeline ==========
            # phases per pair p:  xt DMA (step p-2) -> qk/v matmuls
            # (step p-1) -> scores+exp (step p) -> Z/PV/proj (step p+1).
            # Every matmul emitted has its inputs ready a full step in
            # advance, so the Tensor queue never head-of-line blocks and
            # the PE stays dense (HAM keeps the high clock).
            mwx_sbs = [None] * BPC
            xt_sbs = {}
            qk_sbs = {}
            v_sbs = {}
            expT_sbs = {}
            esum_sbs = {}
            atn_sbs = {}

            def xt_dma(p, b):
                xt_sb = xp.tile([128, 2, 2, S], BF16, tag="xt")
                for ti in range(2):
                    nc.sync.dma_start(
                        out=xt_sb[:, :, ti, :],
                        in_=xt_d[b, 2 * p + ti].rearrange(
                            "(cc q) s -> q cc s", q=128))
                xt_sbs[(p, b)] = xt_sb

            def qk_mms(p, b):
                xt_sb = xt_sbs[(p, b)]
                qk_sb = qp.tile([128, 4, 2 * S], BF16, tag="qk")
                for jb in range(4):
                    qps = pp1.tile([128, 2 * S], F32, tag="p1")
                    for cc in range(2):
                        nc.tensor.matmul(
                            qps,
                            lhsT=wqk_sb[:, cc, jb * 128:(jb + 1) * 128],
                            rhs=xt_sb[:, cc, :, :],
                            start=(cc == 0), stop=(cc == 1))
                    nc.vector.tensor_tensor(
                        out=qk_sb[:, jb, :], in0=qps,
                        in1=bqk_sb[:, jb].unsqueeze(-1).to_broadcast([128, 2 * S]),
                        op=AL.add)
                qk_sbs[(p, b)] = qk_sb

            def v_mms(p, b):
                xt_sb = xt_sbs.pop((p, b))
                for ti in range(2):
                    v_sb = vp.tile([128, 2, C], BF16, tag="v")
                    vps = pp1.tile([128, 2, C], F32, tag="p1")
                    for sb_ in range(2):
                        for cc in range(2):
                            nc.tensor.matmul(
                                vps[:, sb_, :],
                                lhsT=xt_sb[:, cc, ti,
                                           sb_ * 128:sb_ * 128 + 128],
                                rhs=wv_sb[:, cc, :],
                                start=(sb_ == 0 and cc == 0),
                                stop=(sb_ == 1 and cc == 1))
                    nc.vector.tensor_copy(out=v_sb, in_=vps)
                    v_sbs[(2 * p + ti, b)] = v_sb

            def scores(p, b, ti, jbq, rpp):
                t = 2 * p + ti
                toff = ti * S
                qk_sb = qk_sbs[(p, b)]
                mwx_sb = mwx_sbs[b]
                if (t, b) not in expT_sbs:
                    expT_sbs[(t, b)] = ep.tile([128, 2, 4, 2 * S], BF16,
                                               tag="expT", name="expT")
                expT = expT_sbs[(t, b)]
                sc_ps = psc.tile([128, 2, 2 * S], F32, tag="sc")
                for rr in range(2):
                    rg = 2 * rpp + rr
                    for kb in range(2):
                        nc.tensor.matmul(
                            sc_ps[:, rr, kb * S:(kb + 1) * S],
                            lhsT=qk_sb[32 * rg:32 * rg + 32, 2 + jbq,
                                       toff + kb * 128:toff + kb * 128 + 128],
                            rhs=qk_sb[32 * rg:32 * rg + 32, jbq,
                                      toff:toff + S],
                            start=(kb == 0), stop=False,
                            skip_group_check=True,
                            tile_position=(32 * rg, 0))
                    nc.tensor.matmul(
                        sc_ps[:, rr, :],
                        lhsT=mwx_sb[32 * rg:32 * rg + 16, jbq, t, :],
                        rhs=e8r2_sb[32 * rg:32 * rg + 16, :],
                        start=False, stop=True,
                        skip_group_check=True,
                        tile_position=(32 * rg, 0))
                nc.scalar.activation(
                    out=expT[:, jbq, 2 * rpp:2 * rpp + 2, :],
                    in_=sc_ps, func=ACTF.Exp, scale=SCALE)

            def esum(p, b, ti):
                # pre-sum the exp key halves so Z needs one N=256 matmul
                # per (head-group, quad) and reciprocal reads PSUM direct
                t = 2 * p + ti
                expT = expT_sbs[(t, b)]
                esum_sb = zp.tile([128, 2, 4, S], BF16, tag="esum", bufs=4)
                nc.vector.tensor_tensor(
                    out=esum_sb, in0=expT[:, :, :, 0:S],
                    in1=expT[:, :, :, S:2 * S], op=AL.add)
                esum_sbs[(t, b)] = esum_sb

            def rest_tile(p, b, ti):
                t = 2 * p + ti
                expT = expT_sbs.pop((t, b))
                esum_sb = esum_sbs.pop((t, b))
                v_sb = v_sbs.pop((t, b))
                if ti == 0:
                    atn_sbs[(p, b)] = ap_.tile([128, 2, 2, S], BF16,
                                               tag="atn", name="atn")
                atn_sb = atn_sbs[(p, b)]

                zrep = pp1.tile([128, 2, S], F32, tag="p1")
                for jbq in range(2):
                    for rg in range(4):
                        nc.tensor.matmul(
                            zrep[32 * rg:32 * rg + 32, jbq, :],
                            lhsT=ones32_sb,
                            rhs=esum_sb[:, jbq, rg, :],
                            start=True, stop=True,
                            skip_group_check=True,
                            tile_position=(0, 32 * rg))

                at = pp1.tile([128, 2, S], F32, tag="p1")
                for jbq in range(2):
                    for rg in range(4):
                        hh = 4 * jbq + rg
                        for kb in range(2):
                            nc.tensor.matmul(
                                at[32 * rg:32 * rg + 32, jbq, :],
                                lhsT=v_sb[:, kb, 32 * hh:32 * hh + 32],
                                rhs=expT[:, jbq, rg, kb * S:(kb + 1) * S],
                                start=(jbq == 0 and kb == 0),
                                stop=(jbq == 1 and kb == 1),
                                skip_group_check=True,
                                tile_position=(0, 32 * rg))

                zinv_sb = zp.tile([128, 2, S], F32, tag="zinv")
                nc.vector.reciprocal_approx_fast(out=zinv_sb, in_=zrep)
                nc.vector.tensor_tensor(out=atn_sb[:, :, ti, :], in0=at,
                                        in1=zinv_sb, op=AL.mult)

            def proj_out(p, b):
                atn_sb = atn_sbs.pop((p, b))
                out_sb = op_.tile([128, 2, 2 * S], BF16, tag="out")
                for jb in range(2):
                    po = pp1.tile([128, 2 * S], F32, tag="p1")
                    for cc in range(2):
                        nc.tensor.matmul(
                            po,
                            lhsT=wp_sb[:, cc, jb * 128:(jb + 1) * 128],
                            rhs=atn_sb[:, cc, :, :],
                            start=(cc == 0), stop=(cc == 1))
                    nc.vector.tensor_tensor(
                        out=out_sb[:, jb, :], in0=po,
                        in1=bp_sb[:, jb].unsqueeze(-1).to_broadcast([128, 2 * S]),
                        op=AL.add)
                nc.sync.dma_start(out=out_d[b, p], in_=out_sb)

            GROUPS = ((0, 0), (0, 1), (1, 0), (1, 1))

            def emit_scores(p, b):
                for ti in range(2):
                    for jbq, rpp in GROUPS:
                        scores(p, b, ti, jbq, rpp)
                esum(p, b, 0)
                esum(p, b, 1)

            # ---- preamble ----
            mwx_sbs[0] = routing(0)
            for b in range(BPC):
                xt_dma(0, b)
                if NP > 1:
                    xt_dma(1, b)
            for b in range(BPC):
                qk_mms(0, b)
                v_mms(0, b)
            # step 0: batch-0 scores run while batch-1's routing matmuls
            # fill the Tensor queue behind them
            for b in range(BPC):
                if 2 < NP:
                    xt_dma(2, b)
                if 1 < NP:
                    qk_mms(1, b)
            emit_scores(0, 0)
            mwx_sbs[1] = routing(1)
            for b in range(BPC):
                if 1 < NP:
                    v_mms(1, b)
            emit_scores(0, 1)

            # ---- steady state ----
            for s in range(1, NP):
                for b in range(BPC):
                    if s + 2 < NP:
                        xt_dma(s + 2, b)
                    if s + 1 < NP:
                        qk_mms(s + 1, b)
                for b in range(BPC):
                    rest_tile(s - 1, b, 0)
                    rest_tile(s - 1, b, 1)
                    proj_out(s - 1, b)
                for b in range(BPC):
                    if s + 1 < NP:
                        v_mms(s + 1, b)
                for ti in range(2):
                    for jbq, rpp in GROUPS:
                        for b in range(BPC):
                            scores(s, b, ti, jbq, rpp)
                for b in range(BPC):
                    esum(s, b, 0)
                    esum(s, b, 1)

            # ---- tail ----
            for b in range(BPC):
                rest_tile(NP - 1, b, 0)
                rest_tile(NP - 1, b, 1)
                proj_out(NP - 1, b)

    nc.compile()
    return nc


def _host_prep(x, w_qkv, b_qkv, w_proj, b_proj):
    bf16 = ml_dtypes.bfloat16
    x4 = x.reshape(B, T, S, C)
    xt = np.ascontiguousarray(x4.transpose(0, 1, 3, 2)).astype(bf16)
    xsum = x4.reshape(B, T, NW, WIN, C).sum(3, dtype=np.float64).astype(np.float32)
    xsumt = np.ascontiguousarray(xsum.transpose(0, 3, 1, 2))  # [B, C, T, NW]

    # fold v bias through the projection: out = atn@Wp + (bp + bv@Wp)
    bp_eff = (b_proj + b_qkv[2 * C:] @ w_proj).astype(np.float32)

    shared = {
        "wqk_bf": np.ascontiguousarray(w_qkv[:, :2 * C]).astype(bf16),
        "wqk_f32": np.ascontiguousarray(w_qkv[:, :2 * C]).astype(np.float32),
        "wv_bf": np.ascontiguousarray(w_qkv[:, 2 * C:]).astype(bf16),
        "wproj_bf": w_proj.astype(bf16),
        "bqk_cols": np.ascontiguousarray(
            b_qkv[:2 * C].reshape(4, 128).T).astype(np.float32),
        "bqk_reg": np.ascontiguousarray(
            (WIN * b_qkv[:2 * C]).reshape(4, 128).T).astype(np.float32),
        "bp_col": np.ascontiguousarray(bp_eff.reshape(2, 128).T),
        "e8r2": _make_e8r2(),
    }
    in_maps = []
    for core in range(NCORES):
        b0 = core * BPC
        m = dict(shared)
        m["xt"] = np.ascontiguousarray(xt[b0:b0 + BPC])
        m["xsumt"] = np.ascontiguousarray(xsumt[b0:b0 + BPC])
        in_maps.append(m)
    return in_maps


def _make_e8r2():
    e = np.zeros((128, 2 * S), ml_dtypes.bfloat16)
    q = np.arange(S) // WIN  # query window of column q
    for rg in range(4):
        for u in range(2):
            for w in range(NW):
                e[32 * rg + 8 * u + w, u * S:(u + 1) * S][q == w] = 1.0
    return e


def kernel(x, w_qkv, b_qkv, w_proj, b_proj, **_unused_scalars):
    x = np.asarray(x, dtype=np.float32)
    w_qkv = np.asarray(w_qkv, dtype=np.float32)
    b_qkv = np.asarray(b_qkv, dtype=np.float32)
    w_proj = np.asarray(w_proj, dtype=np.float32)
    b_proj = np.asarray(b_proj, dtype=np.float32)

    if "nc" not in _CACHE:
        _CACHE["nc"] = _build_nc()
    nc = _CACHE["nc"]

    in_maps = _host_prep(x, w_qkv, b_qkv, w_proj, b_proj)
    res = run_bass_kernel_spmd(nc, in_maps, core_ids=list(range(NCORES)))

    out = np.empty((B, NP, 128, 2, 2 * S), np.float32)
    for core in range(NCORES):
        out[core * BPC:(core + 1) * BPC] = res.results[core]["out"]
    # [B, pair, p128, jb, (ti s)] -> [B, N, C]: token = pair*512 + tis,
    # feature c = jb*128 + p128
    out = out.transpose(0, 1, 4, 3, 2).reshape(B, T * S, C)
    return np.ascontiguousarray(out)


# revision 6
# speedup vs baseline: 1.2770x; 1.0552x over previous
"""BiLevelRoutingAttention Trainium2 kernel (v3).

The Tensor-queue is the bottleneck: span ~= sum of LDWEIGHTS+issue per
matmul (~98ns each in v2, 3123 matmuls -> 305us).  v3 cuts the matmul
count per (b,t) tile from 82 to 58 and the routing preamble from ~264
to ~40 matmuls per batch:

  - qk projection pair-batched over 2 tiles (N=512): 8 -> 4 MM/tile.
  - out projection feature-major + pair-batched:      4 -> 2 MM/tile.
  - V bias folded into the proj bias on host (out = (PV + bv*Z)/Z @ Wp
    + bp == atn @ Wp + (bp + bv@Wp)): bias matmuls gone.
  - mask add merged over both key halves (K=16 stationary, N=512
    two-block e8r2 constant): 16 -> 8 MM/tile.
  - Z via col-tiled ones (M=32) with N=512 (both key halves in free),
    halves summed on DVE: 16 -> 8 MM/tile.
  - routing sim as block-diagonal fp32 matmuls (M=128 covering all 4
    heads x (2 dup x 8 qwin)): 256 -> 32 MM/batch, and the duplicated
    rows directly provide the K=16 merged-mask stationary layout.
  - mask window-expansion done once per batch by DMA (SBUF->SBUF
    broadcast reads), freeing gpsimd and the per-tile critical path.
  - bf16 output, halves the store DMA.
"""

import sys

sys.path.insert(0, "/opt/trn_rl_repo")

import numpy as np
import ml_dtypes

import concourse.bass as bass
import concourse.bacc as bacc
import concourse.mybir as mybir
import concourse.tile as tile
from concourse.bass_utils import run_bass_kernel_spmd

BF16 = mybir.dt.bfloat16
F32 = mybir.dt.float32

NCORES = 8
B, T, S, C = 16, 16, 256, 256
NW, WIN, NH, D, TK = 8, 32, 8, 32, 4
BPC = B // NCORES  # batches per core
NP = T // 2        # tile pairs per batch
SCALE = float(D) ** -0.5
MASKVAL = -1e9

_CACHE = {}


def _build_nc(nt=T):
    nc = bacc.Bacc("TRN2", target_bir_lowering=False, debug=False)
    AL = mybir.AluOpType
    ACTF = mybir.ActivationFunctionType

    xt_d = nc.dram_tensor("xt", [BPC, nt, C, S], BF16, kind="ExternalInput")
    xs_d = nc.dram_tensor("xsumt", [BPC, C, nt, NW], F32, kind="ExternalInput")
    wqk_d = nc.dram_tensor("wqk_bf", [C, 2 * C], BF16, kind="ExternalInput")
    wqkf_d = nc.dram_tensor("wqk_f32", [C, 2 * C], F32, kind="ExternalInput")
    wv_d = nc.dram_tensor("wv_bf", [C, C], BF16, kind="ExternalInput")
    wp_d = nc.dram_tensor("wproj_bf", [C, C], BF16, kind="ExternalInput")
    bqk_d = nc.dram_tensor("bqk_cols", [128, 4], F32, kind="ExternalInput")
    bqkr_d = nc.dram_tensor("bqk_reg", [128, 4], F32, kind="ExternalInput")
    bp_d = nc.dram_tensor("bp_col", [128, 2], F32, kind="ExternalInput")
    e8r2_d = nc.dram_tensor("e8r2", [128, 2 * S], BF16, kind="ExternalInput")
    # out: [b, pair, feat_part, jb, (ti,s)] bf16 (feature-major)
    out_d = nc.dram_tensor("out", [BPC, NP, 128, 2, 2 * S], BF16,
                           kind="ExternalOutput")

    with tile.TileContext(nc) as tc:
        with (
            tc.tile_pool(name="wpool", bufs=1) as wp,
            tc.tile_pool(name="route", bufs=1) as rp,
            tc.tile_pool(name="xpool", bufs=6) as xp,
            tc.tile_pool(name="qkpool", bufs=4) as qp,
            tc.tile_pool(name="vpool", bufs=8) as vp,
            tc.tile_pool(name="exps", bufs=4) as ep,
            tc.tile_pool(name="zpool", bufs=2) as zp,
            tc.tile_pool(name="apool", bufs=2) as ap_,
            tc.tile_pool(name="opool", bufs=2) as op_,
            tc.tile_pool(name="sc", bufs=2, space="PSUM") as psc,
            tc.tile_pool(name="p1", bufs=4, space="PSUM") as pp1,
        ):
            # PE warm-up first, against junk (no DMA dependency): ramps the
            # HAM clock gate while the weight DMAs are still in flight
            junk_sb = wp.tile([128, 2 * S], BF16)
            nc.vector.memset(junk_sb, 0.5)
            warm_ps = psc.tile([128, 2, 2 * S], F32, tag="sc")
            for w in range(24):
                nc.tensor.matmul(warm_ps[:, 0, :],
                                 lhsT=junk_sb[:, 0:128],
                                 rhs=junk_sb,
                                 start=(w == 0), stop=(w == 23))

            # ---- weights / constants (loaded once) ----
            wqk_sb = wp.tile([128, 2, 2 * C], BF16)
            nc.sync.dma_start(out=wqk_sb, in_=wqk_d.ap().rearrange("(cc p) j -> p cc j", p=128))
            wqkf_sb = wp.tile([128, 2, 2 * C], F32)
            nc.sync.dma_start(out=wqkf_sb, in_=wqkf_d.ap().rearrange("(cc p) j -> p cc j", p=128))
            wv_sb = wp.tile([128, 2, C], BF16)
            nc.sync.dma_start(out=wv_sb, in_=wv_d.ap().rearrange("(cc p) j -> p cc j", p=128))
            wp_sb = wp.tile([128, 2, C], BF16)
            nc.sync.dma_start(out=wp_sb, in_=wp_d.ap().rearrange("(cc p) j -> p cc j", p=128))
            bqk_sb = wp.tile([128, 4], F32)
            nc.sync.dma_start(out=bqk_sb, in_=bqk_d.ap())
            bqkr_sb = wp.tile([128, 4], F32)
            nc.sync.dma_start(out=bqkr_sb, in_=bqkr_d.ap())
            bp_sb = wp.tile([128, 2], F32)
            nc.sync.dma_start(out=bp_sb, in_=bp_d.ap())
            e8r2_sb = wp.tile([128, 2 * S], BF16)
            nc.sync.dma_start(out=e8r2_sb, in_=e8r2_d.ap())
            ones32_sb = wp.tile([128, 32], BF16)
            nc.vector.memset(ones32_sb, 1.0)

            # ================= routing preamble (per batch) =================
            sim_sbs = {}

            def routing_mm(b):
                xs_sb = rp.tile([128, 2, nt * NW], F32, tag=f"xsb{b}")
                nc.sync.dma_start(
                    out=xs_sb,
                    in_=xs_d[b].rearrange("(cc p) t n -> p cc (t n)", p=128))

                # region features: [q;k]^T = Wqk^T @ xsum  (exact fp32)
                rs_ps = pp1.tile([128, 4, nt * NW], F32, tag="p1")
                for jb in range(4):
                    for cc in range(2):
                        nc.tensor.matmul(rs_ps[:, jb, :],
                                         lhsT=wqkf_sb[:, cc, jb * 128:(jb + 1) * 128],
                                         rhs=xs_sb[:, cc, :],
                                         start=(jb == 0 and cc == 0),
                                         stop=(jb == 3 and cc == 1))
                rgs_sb = rp.tile([128, 4, nt * NW], F32, tag=f"rgs{b}")
                nc.vector.tensor_tensor(
                    out=rgs_sb, in0=rs_ps,
                    in1=bqkr_sb[:].unsqueeze(-1).to_broadcast([128, 4, nt * NW]),
                    op=AL.add)

                # block-diagonal q-region stationary: ddiag[32rg+d, jbq, t,
                # 32rg + 8u + qwin] = qreg_rg[d, qwin] (u-duplicated),
                # zeros elsewhere -> one M=128 fp32 matmul per (jbq, t)
                # computes all 4 heads' 8x8 sims with 2x row duplication.
                ddiag = rp.tile([128, 2, nt, 128], F32, tag=f"ddiag{b}")
                nc.vector.memset(ddiag, 0.0)
                for jbq in range(2):
                    for rg in range(4):
                        src = rgs_sb[32 * rg:32 * rg + 32, jbq, :] \
                            .rearrange("p (t n) -> p t n", n=NW) \
                            .unsqueeze(2).to_broadcast([32, nt, 2, NW])
                        nc.vector.tensor_copy(
                            out=ddiag[32 * rg:32 * rg + 32, jbq, :,
                                      32 * rg:32 * rg + 16]
                                .rearrange("p t (u n) -> p t u n", n=NW),
                            in_=src)

                sim_ps = pp1.tile([128, 2, nt * NW], F32, tag="p1")
                for jbq in range(2):
                    for t in range(nt):
                        nc.tensor.matmul(
                            sim_ps[:, jbq, t * NW:(t + 1) * NW],
                            lhsT=ddiag[:, jbq, t, :],
                            rhs=rgs_sb[:, 2 + jbq, t * NW:(t + 1) * NW],
                            start=True, stop=True)
                sim_sb = rp.tile([128, 2, nt * NW], F32, tag=f"sim{b}")
                nc.vector.tensor_copy(out=sim_sb, in_=sim_ps)
                sim_sbs[b] = sim_sb

            def routing_mask(b):
                sim_sb = sim_sbs.pop(b)
                # top-4 threshold per (head, qwin): 4th largest of the 8
                mx_sb = rp.tile([128, 2, nt * NW], F32, tag=f"mx{b}")
                mw_sb = rp.tile([128, 2, nt * NW], BF16, tag=f"mw{b}")
                for t in range(nt):
                    for jbq in range(2):
                        nc.vector.max(
                            out=mx_sb[:, jbq, t * NW:(t + 1) * NW],
                            in_=sim_sb[:, jbq, t * NW:(t + 1) * NW])
                for jbq in range(2):
                    nc.vector.tensor_tensor(
                        out=mw_sb[:, jbq, :]
                            .rearrange("p (t n) -> p t n", n=NW),
                        in0=sim_sb[:, jbq, :]
                            .rearrange("p (t n) -> p t n", n=NW),
                        in1=mx_sb[:, jbq, 3::NW]
                            .unsqueeze(-1).to_broadcast([128, nt, NW]),
                        op=AL.is_ge)
                # mask = (m01 - 1) * 1e9  ->  0 selected / -1e9 dropped
                nc.vector.tensor_scalar(
                    out=mw_sb, in0=mw_sb,
                    scalar1=1.0, scalar2=-MASKVAL,
                    op0=AL.subtract, op1=AL.mult)

                # window-expand the mask for all tiles: first compact the
                # 4 relevant kwins per u-row-group (8 tiny DVE copies),
                # then one DMA broadcast-read expands kwin -> 32 keys:
                # mwx[32rg+8u+qwin, jbq, t, k] =
                #   mw[32rg+8u+qwin, jbq, t*8 + 4u + k//32]
                mw2_sb = rp.tile([128, 2, nt, 4], BF16, tag=f"mw2{b}")
                nc.vector.memset(mw2_sb, 0.0)
                # 32 tiny SBUF->SBUF DMAs, spread over two otherwise-idle
                # engine queues so they drain in parallel
                for rg in range(4):
                    for u in range(2):
                        r0 = 32 * rg + 8 * u
                        for jbq in range(2):
                            eng = nc.scalar if (rg + u + jbq) % 2 else nc.gpsimd
                            eng.dma_start(
                                out=mw2_sb[r0:r0 + 8, jbq, :, :],
                                in_=mw_sb[r0:r0 + 8, jbq, :]
                                    .rearrange("p (t n) -> p t n", n=NW)[
                                        :, :, 4 * u:4 * u + 4])
                mwx_sb = rp.tile([128, 2, nt, 128], BF16, tag=f"mwx{b}")
                nc.vector.tensor_copy(
                    out=mwx_sb.rearrange("p j t (n w) -> p (j t n) w", w=WIN),
                    in_=mw2_sb.rearrange("p j t n -> p (j t n)")
                        .unsqueeze(-1).to_broadcast([128, 2 * nt * 4, WIN]))
                return mwx_sb

            # ================= main loop: 1-pair software pipeline ==========
            # phases per pair p:  xt DMA (step p-2) -> qk/v matmuls
            # (step p-1) -> scores+exp (step p) -> Z/PV/proj (step p+1).
            # Every matmul emitted has its inputs ready a full step in
            # advance, so the Tensor queue never head-of-line blocks and
            # the PE stays dense (HAM keeps the high clock).
            mwx_sbs = [None] * BPC
            xt_sbs = {}
            qk_sbs = {}
            v_sbs = {}
            expT_sbs = {}
            esum_sbs = {}
            atn_sbs = {}

            def xt_dma(p, b):
                xt_sb = xp.tile([128, 2, 2, S], BF16, tag="xt")
                for ti in range(2):
                    nc.sync.dma_start(
                        out=xt_sb[:, :, ti, :],
                        in_=xt_d[b, 2 * p + ti].rearrange(
                            "(cc q) s -> q cc s", q=128))
                xt_sbs[(p, b)] = xt_sb

            def qk_mms(p, b):
                xt_sb = xt_sbs[(p, b)]
                qk_sb = qp.tile([128, 4, 2 * S], BF16, tag="qk")
                for jb in range(4):
                    qps = pp1.tile([128, 2 * S], F32, tag="p1")
                    for cc in range(2):
                        nc.tensor.matmul(
                            qps,
                            lhsT=wqk_sb[:, cc, jb * 128:(jb + 1) * 128],
                            rhs=xt_sb[:, cc, :, :],
                            start=(cc == 0), stop=(cc == 1))
                    nc.vector.tensor_tensor(
                        out=qk_sb[:, jb, :], in0=qps,
                        in1=bqk_sb[:, jb].unsqueeze(-1).to_broadcast([128, 2 * S]),
                        op=AL.add)
                qk_sbs[(p, b)] = qk_sb

            def v_mms(p, b):
                xt_sb = xt_sbs.pop((p, b))
                for ti in range(2):
                    v_sb = vp.tile([128, 2, C], BF16, tag="v")
                    vps = pp1.tile([128, 2, C], F32, tag="p1")
                    for sb_ in range(2):
                        for cc in range(2):
                            nc.tensor.matmul(
                                vps[:, sb_, :],
                                lhsT=xt_sb[:, cc, ti,
                                           sb_ * 128:sb_ * 128 + 128],
                                rhs=wv_sb[:, cc, :],
                                start=(sb_ == 0 and cc == 0),
                                stop=(sb_ == 1 and cc == 1))
                    nc.vector.tensor_copy(out=v_sb, in_=vps)
                    v_sbs[(2 * p + ti, b)] = v_sb

            def scores(p, b, ti, jbq, rpp):
                t = 2 * p + ti
                toff = ti * S
                qk_sb = qk_sbs[(p, b)]
                mwx_sb = mwx_sbs[b]
                if (t, b) not in expT_sbs:
                    expT_sbs[(t, b)] = ep.tile([128, 2, 4, 2 * S], BF16,
                                               tag="expT", name="expT")
                expT = expT_sbs[(t, b)]
                sc_ps = psc.tile([128, 2, 2 * S], F32, tag="sc")
                for kb in range(2):
                    for rr in range(2):
                        rg = 2 * rpp + rr
                        nc.tensor.matmul(
                            sc_ps[:, rr, kb * S:(kb + 1) * S],
                            lhsT=qk_sb[32 * rg:32 * rg + 32, 2 + jbq,
                                       toff + kb * 128:toff + kb * 128 + 128],
                            rhs=qk_sb[32 * rg:32 * rg + 32, jbq,
                                      toff:toff + S],
                            start=(kb == 0), stop=False,
                            skip_group_check=True,
                            tile_position=(32 * rg, 0))
                for rr in range(2):
                    rg = 2 * rpp + rr
                    nc.tensor.matmul(
                        sc_ps[:, rr, :],
                        lhsT=mwx_sb[32 * rg:32 * rg + 16, jbq, t, :],
                        rhs=e8r2_sb[32 * rg:32 * rg + 16, :],
                        start=False, stop=True,
                        skip_group_check=True,
                        tile_position=(32 * rg, 0))
                nc.scalar.activation(
                    out=expT[:, jbq, 2 * rpp:2 * rpp + 2, :],
                    in_=sc_ps, func=ACTF.Exp, scale=SCALE)

            def esum(p, b, ti):
                # pre-sum the exp key halves so Z needs one N=256 matmul
                # per (head-group, quad) and reciprocal reads PSUM direct
                t = 2 * p + ti
                expT = expT_sbs[(t, b)]
                esum_sb = zp.tile([128, 2, 4, S], BF16, tag="esum", bufs=4)
                nc.vector.tensor_tensor(
                    out=esum_sb, in0=expT[:, :, :, 0:S],
                    in1=expT[:, :, :, S:2 * S], op=AL.add)
                esum_sbs[(t, b)] = esum_sb

            def rest_tile(p, b, ti):
                t = 2 * p + ti
                expT = expT_sbs.pop((t, b))
                esum_sb = esum_sbs.pop((t, b))
                v_sb = v_sbs.pop((t, b))
                if ti == 0:
                    atn_sbs[(p, b)] = ap_.tile([128, 2, 2, S], BF16,
                                               tag="atn", name="atn")
                atn_sb = atn_sbs[(p, b)]

                zrep = pp1.tile([128, 2, S], F32, tag="p1")
                for rg in range(4):
                    nc.tensor.matmul(
                        zrep[32 * rg:32 * rg + 32, :, :],
                        lhsT=ones32_sb,
                        rhs=esum_sb[:, :, rg, :],
                        start=True, stop=True,
                        skip_group_check=True,
                        tile_position=(0, 32 * rg))

                at = pp1.tile([128, 2, S], F32, tag="p1")
                for jbq in range(2):
                    for kb in range(2):
                        for rg in range(4):
                            hh = 4 * jbq + rg
                            nc.tensor.matmul(
                                at[32 * rg:32 * rg + 32, jbq, :],
                                lhsT=v_sb[:, kb, 32 * hh:32 * hh + 32],
                                rhs=expT[:, jbq, rg, kb * S:(kb + 1) * S],
                                start=(jbq == 0 and kb == 0),
                                stop=(jbq == 1 and kb == 1),
                                skip_group_check=True,
                                tile_position=(0, 32 * rg))

                zinv_sb = zp.tile([128, 2, S], F32, tag="zinv")
                nc.vector.reciprocal_approx_fast(out=zinv_sb, in_=zrep)
                nc.vector.tensor_tensor(out=atn_sb[:, :, ti, :], in0=at,
                                        in1=zinv_sb, op=AL.mult)

            def proj_out(p, b):
                atn_sb = atn_sbs.pop((p, b))
                out_sb = op_.tile([128, 2, 2 * S], BF16, tag="out")
                for jb in range(2):
                    po = pp1.tile([128, 2 * S], F32, tag="p1")
                    for cc in range(2):
                        nc.tensor.matmul(
                            po,
                            lhsT=wp_sb[:, cc, jb * 128:(jb + 1) * 128],
                            rhs=atn_sb[:, cc, :, :],
                            start=(cc == 0), stop=(cc == 1))
                    nc.vector.tensor_tensor(
                        out=out_sb[:, jb, :], in0=po,
                        in1=bp_sb[:, jb].unsqueeze(-1).to_broadcast([128, 2 * S]),
                        op=AL.add)
                nc.sync.dma_start(out=out_d[b, p], in_=out_sb)

            GROUPS = ((0, 0), (0, 1), (1, 0), (1, 1))

            def emit_scores(p, b):
                for ti in range(2):
                    for jbq, rpp in GROUPS:
                        scores(p, b, ti, jbq, rpp)
                esum(p, b, 0)
                esum(p, b, 1)

            # ---- preamble: zip both batches' routing matmul phases with
            # the first pairs' projection matmuls, then the (DVE/DMA-only)
            # mask phases, so the Tensor queue never drains ----
            for b in range(BPC):
                xt_dma(0, b)
                if NP > 1:
                    xt_dma(1, b)
            routing_mm(0)
            qk_mms(0, 0)
            v_mms(0, 0)
            routing_mm(1)
            qk_mms(0, 1)
            v_mms(0, 1)
            mwx_sbs[0] = routing_mask(0)
            mwx_sbs[1] = routing_mask(1)
            for b in range(BPC):
                if 2 < NP:
                    xt_dma(2, b)
                if 1 < NP:
                    qk_mms(1, b)

            # ---- steady state: 16 score groups per step, zipped with
            # rest/proj/qk/v chunks so every ACT wait is covered by
            # independent Tensor work ----
            def sc4(p, ti, jbq, rpp):
                scores(p, 0, ti, jbq, rpp)
                scores(p, 1, ti, jbq, rpp)

            for s in range(NP):
                q = s - 1
                have_rest = q >= 0
                have_next = s + 1 < NP
                for b in range(BPC):
                    if s + 2 < NP:
                        xt_dma(s + 2, b)

                sc4(s, 0, 0, 0)
                if have_rest:
                    rest_tile(q, 0, 0)
                sc4(s, 0, 0, 1)
                if have_rest:
                    rest_tile(q, 1, 0)
                sc4(s, 0, 1, 0)
                if have_rest:
                    rest_tile(q, 0, 1)
                sc4(s, 0, 1, 1)
                if have_rest:
                    rest_tile(q, 1, 1)
                sc4(s, 1, 0, 0)
                if have_rest:
                    proj_out(q, 0)
                if have_next:
                    qk_mms(s + 1, 0)
                sc4(s, 1, 0, 1)
                if have_rest:
                    proj_out(q, 1)
                if have_next:
                    qk_mms(s + 1, 1)
                sc4(s, 1, 1, 0)
                if have_next:
                    v_mms(s + 1, 0)
                sc4(s, 1, 1, 1)
                if have_next:
                    v_mms(s + 1, 1)
                for b in range(BPC):
                    esum(s, b, 0)
                    esum(s, b, 1)

            # ---- tail ----
            for b in range(BPC):
                rest_tile(NP - 1, b, 0)
                rest_tile(NP - 1, b, 1)
                proj_out(NP - 1, b)

    nc.compile()
    return nc


def _host_prep(x, w_qkv, b_qkv, w_proj, b_proj):
    bf16 = ml_dtypes.bfloat16
    x4 = x.reshape(B, T, S, C)
    xt = np.ascontiguousarray(x4.transpose(0, 1, 3, 2)).astype(bf16)
    xsum = x4.reshape(B, T, NW, WIN, C).sum(3, dtype=np.float64).astype(np.float32)
    xsumt = np.ascontiguousarray(xsum.transpose(0, 3, 1, 2))  # [B, C, T, NW]

    # fold v bias through the projection: out = atn@Wp + (bp + bv@Wp)
    bp_eff = (b_proj + b_qkv[2 * C:] @ w_proj).astype(np.float32)

    shared = {
        "wqk_bf": np.ascontiguousarray(w_qkv[:, :2 * C]).astype(bf16),
        "wqk_f32": np.ascontiguousarray(w_qkv[:, :2 * C]).astype(np.float32),
        "wv_bf": np.ascontiguousarray(w_qkv[:, 2 * C:]).astype(bf16),
        "wproj_bf": w_proj.astype(bf16),
        "bqk_cols": np.ascontiguousarray(
            b_qkv[:2 * C].reshape(4, 128).T).astype(np.float32),
        "bqk_reg": np.ascontiguousarray(
            (WIN * b_qkv[:2 * C]).reshape(4, 128).T).astype(np.float32),
        "bp_col": np.ascontiguousarray(bp_eff.reshape(2, 128).T),
        "e8r2": _make_e8r2(),
    }
    in_maps = []
    for core in range(NCORES):
        b0 = core * BPC
        m = dict(shared)
        m["xt"] = np.ascontiguousarray(xt[b0:b0 + BPC])
        m["xsumt"] = np.ascontiguousarray(xsumt[b0:b0 + BPC])
        in_maps.append(m)
    return in_maps


def _make_e8r2():
    e = np.zeros((128, 2 * S), ml_dtypes.bfloat16)
    q = np.arange(S) // WIN  # query window of column q
    for rg in range(4):
        for u in range(2):
            for w in range(NW):
                e[32 * rg + 8 * u + w, u * S:(u + 1) * S][q == w] = 1.0
    return e


def kernel(x, w_qkv, b_qkv, w_proj, b_proj, **_unused_scalars):
    x = np.asarray(x, dtype=np.float32)
    w_qkv = np.asarray(w_qkv, dtype=np.float32)
    b_qkv = np.asarray(b_qkv, dtype=np.float32)
    w_proj = np.asarray(w_proj, dtype=np.float32)
    b_proj = np.asarray(b_proj, dtype=np.float32)

    if "nc" not in _CACHE:
        _CACHE["nc"] = _build_nc()
    nc = _CACHE["nc"]

    in_maps = _host_prep(x, w_qkv, b_qkv, w_proj, b_proj)
    res = run_bass_kernel_spmd(nc, in_maps, core_ids=list(range(NCORES)))

    out = np.empty((B, NP, 128, 2, 2 * S), np.float32)
    for core in range(NCORES):
        out[core * BPC:(core + 1) * BPC] = res.results[core]["out"]
    # [B, pair, p128, jb, (ti s)] -> [B, N, C]: token = pair*512 + tis,
    # feature c = jb*128 + p128
    out = out.transpose(0, 1, 4, 3, 2).reshape(B, T * S, C)
    return np.ascontiguousarray(out)
